# revision 22
# baseline (speedup 1.0000x reference)
"""KMeans assignment kernel for Trainium2 (8 NeuronCores, SPMD).

argmin_k ||f_n - c_k||^2 == argmax_k (2*f.c_k - |c_k|^2)  (the |f|^2 term is
row-constant, order-preserving to drop).

Shipped design (mode="flag"): the device computes APPROXIMATE scores with a
single fp16 x fp16 matmul pass (1 cyc/row on the PE; feature tiles are
pre-transposed on the host so the PE runs no transposes, and the -|c|^2 bias
rides in the same PSUM accumulation as a 3-row fp16-ladder matmul, residual
~6e-8).  The DVE max8 op returns the TOP-2 score values per row (not just the
max), so the device also emits each row's top-2 gap.  Rows whose gap is below
FLAG_T (~22 sigma of the fp16 score noise; ~2.5k of 131k rows) are recomputed
exactly in fp64 on the host.  Every returned index is therefore exact: large-
gap rows cannot flip under the bounded device noise, and near-tie rows are
resolved by the host at full precision.  Device loop per 128-row tile:
8 fp16 matmuls (N=512) + 1 bias matmul per PSUM half, then DVE max8 +
max_index; ~2.7us/tile, PE and DVE both ~95% busy.

Other modes kept for reference: "safe" = f32r main pass + two fp16 correction
passes (~2^-23-grade products, no host fixup needed; ~860us).  "fast" = f32r
main + fp8e5m2 DoubleRow correction (1.5 cyc/row, sigma ~5e-4 -> flips 3 rows
on the test seed; not shippable without the flag/fixup mechanism).

Sharding: features split over N across 8 cores (data parallel); centroids
replicated; no cross-core communication.  All operand prep (transpose-tiling,
fp16/f32r/fp8 encodings, bias ladder) happens on the host in prep_inputs().
"""
import sys

sys.path.insert(0, "/opt/trn_rl_repo")

import numpy as np
from contextlib import ExitStack, nullcontext

import concourse.bacc as bacc
import concourse.mybir as mybir
from concourse import tile
from concourse.bass_utils import run_bass_kernel_spmd

N, D, K = 131072, 512, 1024
N_CORES = 8
N_PER_CORE = N // N_CORES          # 16384
N_TILES = N_PER_CORE // 128        # 128 row-tiles per core
ND = D // 128                      # 4 contraction tiles
F32 = mybir.dt.float32
F32R = mybir.dt.float32r
F16 = mybir.dt.float16
F8E5 = mybir.dt.float8e5
U32 = mybir.dt.uint32

_cached = {}

# shipped configuration (see build_bass options)
SHIP_KW = {"mode": "flag", "bias_mode": "pe", "w16": True}
# host-side fixup threshold: rows whose device top-2 gap is below this get an
# exact fp64 recompute.  Device score error is ~1.33e-2 rms (fp16 x fp16
# operands); T=0.30 is ~22 sigma -> P(missed flip) ~ 0.
# Simulated on the test seed: 2516 flagged rows, 0 mismatches, 0 missed.
FLAG_T = 0.30


def build_bass(n_tiles: int = N_TILES, repeat: int = 1,
               mode: str = "safe", bias_mode: str = "pe",
               w16: bool = False):
    n_rows = n_tiles * 128
    nc = bacc.Bacc()
    fw_dt = F16 if w16 else F32R
    fh_p = nc.declare_dram_parameter("fh", [n_rows, D], fw_dt, isOutput=False)
    ch_p = nc.declare_dram_parameter("ch", [128, ND * K], fw_dt, isOutput=False)
    if mode == "flag":
        pass  # main pass only; near-ties resolved on the host
    elif mode == "safe":
        f16_p = nc.declare_dram_parameter("f16", [n_rows, 2 * D], F16, isOutput=False)
        cl16_p = nc.declare_dram_parameter("cl16", [128, ND * K], F16, isOutput=False)
        ch16_p = nc.declare_dram_parameter("ch16", [128, ND * K], F16, isOutput=False)
    else:
        f8_p = nc.declare_dram_parameter("f8", [n_rows, 2 * D], F8E5, isOutput=False)
        c8_p = nc.declare_dram_parameter("c8", [128, 2 * ND * K], F8E5, isOutput=False)
    if bias_mode == "pe":
        nb_p = nc.declare_dram_parameter("nb", [3, K], fw_dt, isOutput=False)
    else:
        nb_p = nc.declare_dram_parameter("nb", [1, K], F32, isOutput=False)
    out_w = 3 if mode == "flag" else 1
    out = nc.declare_dram_parameter("out", [n_rows, out_w], F32, isOutput=True)

    with tile.TileContext(nc) as tc, ExitStack() as ctx:
        const = ctx.enter_context(tc.tile_pool(name="const", bufs=1))
        work = ctx.enter_context(tc.tile_pool(name="work", bufs=3))
        red = ctx.enter_context(tc.tile_pool(name="red", bufs=4))
        psB = ctx.enter_context(tc.tile_pool(name="psB", bufs=3, space="PSUM"))

        # centroid-side operands resident in SBUF
        cht = const.tile([128, ND * K], fw_dt)
        nc.sync.dma_start(out=cht[:], in_=ch_p[:])
        if mode == "safe":
            cl16t = const.tile([128, ND * K], F16)
            ch16t = const.tile([128, ND * K], F16)
            nc.sync.dma_start(out=cl16t[:], in_=cl16_p[:])
            nc.sync.dma_start(out=ch16t[:], in_=ch16_p[:])
        elif mode == "fast":
            c8t = const.tile([128, 2 * ND * K], F8E5)
            nc.sync.dma_start(out=c8t[:], in_=c8_p[:])

        if bias_mode == "pe":
            nbt = const.tile([3, K], fw_dt)
            nc.sync.dma_start(out=nbt[:], in_=nb_p[:])
            ones3 = const.tile([3, 128], F16 if w16 else F32)
            nc.vector.memset(ones3[:], 1.0)
        else:
            nb1 = const.tile([1, K], F32)
            nc.sync.dma_start(out=nb1[:], in_=nb_p[:])
            ncsq_b = const.tile([128, K], F32)
            nc.gpsimd.partition_broadcast(ncsq_b[:], nb1[:])

        idx8 = const.tile([128, n_tiles * 8], U32, tag="idx8")
        fbuf = const.tile([128, n_tiles], F32)
        if mode == "flag":
            m018 = const.tile([128, n_tiles * 8], F32, tag="m018")

        loop_ctx = tc.For_i(0, repeat, 1) if repeat > 1 else nullcontext()
        with loop_ctx:
            for rt in range(n_tiles):
                rs = slice(rt * 128, (rt + 1) * 128)
                ft = work.tile([128, D], fw_dt, tag="ft")
                nc.sync.dma_start(out=ft[:], in_=fh_p[rs, :])
                if mode == "safe":
                    f16t = work.tile([128, 2 * D], F16, tag="f16t")
                    nc.sync.dma_start(out=f16t[:], in_=f16_p[rs, :])
                elif mode == "fast":
                    f8t = work.tile([128, 2 * D], F8E5, tag="f8t")
                    nc.sync.dma_start(out=f8t[:], in_=f8_p[rs, :])

                mp = psB.tile([128, K], F32, tag="mp")
                # main pass: h_f @ h_c, f32r, d-outer so each weight loads once
                for d in range(ND):
                    for kh in range(2):
                        ks = slice(kh * 512, (kh + 1) * 512)
                        nc.tensor.matmul(
                            mp[:, ks],
                            lhsT=ft[:, d * 128:(d + 1) * 128],
                            rhs=cht[:, d * K + kh * 512:d * K + (kh + 1) * 512],
                            start=d == 0, stop=False)
                if mode == "safe":
                    # corrections: fp16(f) @ fp16(l_c), then fp16(l_f) @ fp16(c)
                    for q, cv in enumerate((cl16t, ch16t)):
                        for d in range(ND):
                            for kh in range(2):
                                ks = slice(kh * 512, (kh + 1) * 512)
                                last = q == 1 and d == ND - 1
                                nc.tensor.matmul(
                                    mp[:, ks],
                                    lhsT=f16t[:, q * D + d * 128:q * D + (d + 1) * 128],
                                    rhs=cv[:, d * K + kh * 512:d * K + (kh + 1) * 512],
                                    start=False,
                                    stop=last and bias_mode != "pe")
                elif mode == "fast":
                    # fp8e5m2 DoubleRow: e5m2(f)@e5m2(l_c) + e5m2(l_f)@e5m2(c)
                    for d in range(ND):
                        lw = f8t[:, 2 * d * 128:(2 * d + 2) * 128].rearrange(
                            "p (q n) -> p q n", q=2)
                        cw = c8t[:, 2 * d * K:(2 * d + 2) * K].rearrange(
                            "p (q h k) -> p q h k", q=2, h=2)
                        for kh in range(2):
                            ks = slice(kh * 512, (kh + 1) * 512)
                            last = d == ND - 1
                            nc.tensor.matmul(
                                mp[:, ks],
                                lhsT=lw,
                                rhs=cw[:, :, kh, :],
                                perf_mode=mybir.MatmulPerfMode.DoubleRow,
                                start=False,
                                stop=last and bias_mode != "pe")
                if bias_mode == "pe":
                    # bias -|c|^2 as a 3-row f32r matmul: [1,1,1] x [nh;nl;nll]
                    for kh in range(2):
                        ks = slice(kh * 512, (kh + 1) * 512)
                        nc.tensor.matmul(
                            mp[:, ks],
                            lhsT=ones3[:] if w16 else ones3[:].bitcast(F32R),
                            rhs=nbt[:, ks],
                            start=False, stop=True)
                    sarg = mp
                else:
                    # GPSIMD has no PSUM port: Act copies PSUM->SBUF, Pool adds
                    s0 = work.tile([128, K], F32, tag="s0")
                    nc.scalar.copy(out=s0[:], in_=mp[:])
                    s = work.tile([128, K], F32, tag="s")
                    nc.gpsimd.tensor_tensor(out=s[:], in0=s0[:], in1=ncsq_b[:],
                                            op=mybir.AluOpType.add)
                    sarg = s

                if mode == "flag":
                    mv = m018[:, rt * 8:(rt + 1) * 8]
                else:
                    mv = red.tile([128, 8], F32, tag="mv")[:]
                nc.vector.max(mv, sarg[:])
                nc.vector.max_index(idx8[:, rt * 8:(rt + 1) * 8], mv, sarg[:])

        # gather col 0 of each 8-block, cast u32 -> f32, store
        nc.vector.tensor_copy(out=fbuf[:], in_=idx8[:, 0:n_tiles * 8:8])
        nc.sync.dma_start(out=out[:, 0].rearrange("(t p) -> p t", p=128),
                          in_=fbuf[:])
        if mode == "flag":
            nc.sync.dma_start(out=out[:, 1].rearrange("(t p) -> p t", p=128),
                              in_=m018[:, 0:n_tiles * 8:8])
            nc.sync.dma_start(out=out[:, 2].rearrange("(t p) -> p t", p=128),
                              in_=m018[:, 1:n_tiles * 8:8])

    nc.finalize()
    return nc


def _rne12(x: np.ndarray) -> np.ndarray:
    """Round fp32 to 12-bit significand (exactly representable in FP22)."""
    u = np.ascontiguousarray(x, dtype=np.float32).view(np.uint32)
    r = (u + np.uint32(0x7FF) + ((u >> np.uint32(12)) & np.uint32(1))) \
        & np.uint32(0xFFFFF000)
    return r.view(np.float32)


def _tile_rows(x: np.ndarray) -> np.ndarray:
    """[R, D] row-major -> [R, D] where row t*128+p holds (d = a*128+p)-major
    data for rows t*128+n: out[t*128+p, a*128+n] = x[t*128+n, a*128+p]."""
    r = x.shape[0]
    t = r // 128
    return np.ascontiguousarray(
        x.reshape(t, 128, ND, 128).transpose(0, 3, 2, 1)).reshape(r, D)


def _tile_cols(x: np.ndarray) -> np.ndarray:
    """[D, K] -> [128, ND*K]: out[p, a*K + k] = x[a*128 + p, k]."""
    return np.ascontiguousarray(
        x.reshape(ND, 128, K).transpose(1, 0, 2)).reshape(128, ND * K)


def prep_inputs(features: np.ndarray, centroids: np.ndarray,
                mode: str = None, bias_mode: str = None, w16: bool = None):
    """Host-side operand prep; returns per-core in_maps for build_bass."""
    import ml_dtypes
    mode = SHIP_KW["mode"] if mode is None else mode
    bias_mode = SHIP_KW["bias_mode"] if bias_mode is None else bias_mode
    w16 = SHIP_KW.get("w16", False) if w16 is None else w16
    f = np.ascontiguousarray(np.asarray(features, dtype=np.float32))
    c = np.ascontiguousarray(np.asarray(centroids, dtype=np.float32))
    c2 = (2.0 * c).astype(np.float32)

    fh = _rne12(f)
    fl = (f - fh).astype(np.float32)
    ch = _rne12(c2)
    cl = (c2 - ch).astype(np.float32)

    fh_t = _tile_rows(f).astype(np.float16) if w16 else _tile_rows(fh)
    ch_t = _tile_cols(c2).astype(np.float16) if w16 else _tile_cols(ch)

    common = {"ch": ch_t}
    if mode == "flag":
        pass
    elif mode == "safe":
        f16pair = np.empty((N, 2 * D), dtype=np.float16)
        f16pair[:, :D] = _tile_rows(f.astype(np.float16).astype(np.float32))
        f16pair[:, D:] = _tile_rows(fl.astype(np.float16).astype(np.float32))
        common["cl16"] = _tile_cols(cl.astype(np.float16).astype(np.float32)
                                    ).astype(np.float16)
        common["ch16"] = _tile_cols(c2.astype(np.float16).astype(np.float32)
                                    ).astype(np.float16)
    else:
        e5 = ml_dtypes.float8_e5m2
        f8pair = np.empty((N, 2 * D), dtype=e5)
        fh8 = f.astype(e5).astype(np.float32)
        fl8 = fl.astype(e5).astype(np.float32)
        ft8 = _tile_rows(fh8)  # q=0: e5m2(f)
        lt8 = _tile_rows(fl8)  # q=1: e5m2(l_f)
        for a in range(ND):
            f8pair[:, (2 * a) * 128:(2 * a + 1) * 128] = \
                ft8[:, a * 128:(a + 1) * 128].astype(e5)
            f8pair[:, (2 * a + 1) * 128:(2 * a + 2) * 128] = \
                lt8[:, a * 128:(a + 1) * 128].astype(e5)
        c8pair = np.empty((128, 2 * ND * K), dtype=e5)
        cl8 = _tile_cols(cl.astype(e5).astype(np.float32))   # q=0: e5m2(l_c)
        ch8 = _tile_cols(c2.astype(e5).astype(np.float32))   # q=1: e5m2(c)
        for a in range(ND):
            c8pair[:, (2 * a) * K:(2 * a + 1) * K] = \
                cl8[:, a * K:(a + 1) * K].astype(e5)
            c8pair[:, (2 * a + 1) * K:(2 * a + 2) * K] = \
                ch8[:, a * K:(a + 1) * K].astype(e5)

    ncsq = -(c.astype(np.float64) ** 2).sum(0, keepdims=True).astype(np.float32)
    if bias_mode == "pe":
        if w16:
            nh = ncsq.astype(np.float16).astype(np.float32)
            nl = (ncsq - nh).astype(np.float16).astype(np.float32)
            nll = (ncsq - nh - nl).astype(np.float16)
            common["nb"] = np.concatenate(
                [nh.astype(np.float16), nl.astype(np.float16), nll], axis=0)
        else:
            nh = _rne12(ncsq)
            nl = _rne12((ncsq - nh).astype(np.float32))
            nll = _rne12((ncsq - nh - nl).astype(np.float32))
            common["nb"] = np.concatenate([nh, nl, nll], axis=0)
    else:
        common["nb"] = ncsq

    in_maps = []
    for cc in range(N_CORES):
        rs = slice(cc * N_PER_CORE, (cc + 1) * N_PER_CORE)
        m = {"fh": fh_t[rs]}
        if mode == "safe":
            m["f16"] = f16pair[rs]
        elif mode == "fast":
            m["f8"] = f8pair[rs]
        m.update(common)
        in_maps.append(m)
    return in_maps


def _get_nc():
    if "nc" not in _cached:
        _cached["nc"] = build_bass(**SHIP_KW)
    return _cached["nc"]


def fixup_flagged(out3: np.ndarray, features: np.ndarray,
                  centroids: np.ndarray) -> np.ndarray:
    """Resolve near-tie rows (device top-2 gap < FLAG_T) exactly in fp64."""
    idx = out3[:, 0].copy()
    gap = out3[:, 1] - out3[:, 2]
    rows = np.flatnonzero(gap < FLAG_T)
    if len(rows):
        f64 = np.asarray(features, dtype=np.float64)[rows]
        c64 = np.asarray(centroids, dtype=np.float64)
        s = 2.0 * (f64 @ c64) - (c64 * c64).sum(0)
        idx[rows] = s.argmax(1).astype(np.float32)
    return idx[:, None].astype(np.float32)


def kernel(features: np.ndarray, centroids: np.ndarray) -> np.ndarray:
    in_maps = prep_inputs(features, centroids)
    nc = _get_nc()
    res = run_bass_kernel_spmd(nc, in_maps, list(range(N_CORES))).results
    out = np.concatenate([res[c]["out"] for c in range(N_CORES)], axis=0)
    if SHIP_KW["mode"] == "flag":
        return fixup_flagged(out, features, centroids)
    return out.astype(np.float32)


def _self_test():
    rng = np.random.default_rng(0)
    f = rng.standard_normal((N, D)).astype(np.float32)
    c = rng.standard_normal((D, K)).astype(np.float32)
    out = kernel(f, c)
    x = f @ c
    ref = (-2 * x + (c * c).sum(0)).argmin(1)
    print("mismatch:", (out[:, 0] != ref).sum(), "/", N)


if __name__ == "__main__":
    _self_test()


# revision 24
# speedup vs baseline: 1.1182x; 1.1182x over previous
"""KMeans assignment kernel for Trainium2 (8 NeuronCores, SPMD).

argmin_k ||f_n - c_k||^2 == argmax_k (2*f.c_k - |c_k|^2)  (the |f|^2 term is
row-constant, order-preserving to drop).

Shipped design (mode="flag"): the device computes APPROXIMATE scores with a
single fp16 x fp16 matmul pass (1 cyc/row on the PE; feature tiles are
pre-transposed on the host so the PE runs no transposes, and the -|c|^2 bias
rides in the same PSUM accumulation as a 3-row fp16-ladder matmul, residual
~6e-8).  The DVE max8 op returns the TOP-2 score values per row (not just the
max), so the device also emits each row's top-2 gap.  Rows whose gap is below
FLAG_T (~22 sigma of the fp16 score noise; ~2.5k of 131k rows) are recomputed
exactly in fp64 on the host.  Every returned index is therefore exact: large-
gap rows cannot flip under the bounded device noise, and near-tie rows are
resolved by the host at full precision.  Device loop per 128-row tile:
8 fp16 matmuls (N=512) + 1 bias matmul per PSUM half, then DVE max8 +
max_index; ~2.7us/tile, PE and DVE both ~95% busy.

Other modes kept for reference: "safe" = f32r main pass + two fp16 correction
passes (~2^-23-grade products, no host fixup needed; ~860us).  "fast" = f32r
main + fp8e5m2 DoubleRow correction (1.5 cyc/row, sigma ~5e-4 -> flips 3 rows
on the test seed; not shippable without the flag/fixup mechanism).

Sharding: features split over N across 8 cores (data parallel); centroids
replicated; no cross-core communication.  All operand prep (transpose-tiling,
fp16/f32r/fp8 encodings, bias ladder) happens on the host in prep_inputs().
"""
import sys

sys.path.insert(0, "/opt/trn_rl_repo")

import numpy as np
from contextlib import ExitStack, nullcontext

import concourse.bacc as bacc
import concourse.mybir as mybir
from concourse import tile
from concourse.bass_utils import run_bass_kernel_spmd

N, D, K = 131072, 512, 1024
N_CORES = 8
N_PER_CORE = N // N_CORES          # 16384
N_TILES = N_PER_CORE // 128        # 128 row-tiles per core
ND = D // 128                      # 4 contraction tiles
F32 = mybir.dt.float32
F32R = mybir.dt.float32r
F16 = mybir.dt.float16
F8E5 = mybir.dt.float8e5
U32 = mybir.dt.uint32

_cached = {}

# shipped configuration (see build_bass options)
SHIP_KW = {"mode": "flag", "bias_mode": "pool", "w16": True}
# host-side fixup threshold: rows whose device top-2 gap is below this get an
# exact fp64 recompute.  Device score error is ~1.33e-2 rms (fp16 x fp16
# operands); T=0.30 is ~22 sigma -> P(missed flip) ~ 0.
# Simulated on the test seed: 2516 flagged rows, 0 mismatches, 0 missed.
FLAG_T = 0.30


def build_bass(n_tiles: int = N_TILES, repeat: int = 1,
               mode: str = "safe", bias_mode: str = "pe",
               w16: bool = False, wide: bool = False):
    n_rows = n_tiles * 128
    nc = bacc.Bacc()
    fw_dt = F16 if w16 else F32R
    fh_p = nc.declare_dram_parameter("fh", [n_rows, D], fw_dt, isOutput=False)
    ch_p = nc.declare_dram_parameter("ch", [128, ND * K], fw_dt, isOutput=False)
    if mode == "flag":
        pass  # main pass only; near-ties resolved on the host
    elif mode == "safe":
        f16_p = nc.declare_dram_parameter("f16", [n_rows, 2 * D], F16, isOutput=False)
        cl16_p = nc.declare_dram_parameter("cl16", [128, ND * K], F16, isOutput=False)
        ch16_p = nc.declare_dram_parameter("ch16", [128, ND * K], F16, isOutput=False)
    else:
        f8_p = nc.declare_dram_parameter("f8", [n_rows, 2 * D], F8E5, isOutput=False)
        c8_p = nc.declare_dram_parameter("c8", [128, 2 * ND * K], F8E5, isOutput=False)
    if bias_mode == "pe":
        nb_p = nc.declare_dram_parameter("nb", [3, K], fw_dt, isOutput=False)
    else:
        nb_p = nc.declare_dram_parameter("nb", [1, K], F32, isOutput=False)
    out_w = 3 if mode == "flag" else 1
    out = nc.declare_dram_parameter("out", [n_rows, out_w], F32, isOutput=True)

    with tile.TileContext(nc) as tc, ExitStack() as ctx:
        const = ctx.enter_context(tc.tile_pool(name="const", bufs=1))
        work = ctx.enter_context(tc.tile_pool(name="work", bufs=3))
        red = ctx.enter_context(tc.tile_pool(name="red", bufs=4))
        psB = ctx.enter_context(tc.tile_pool(name="psB", bufs=3, space="PSUM"))

        # centroid-side operands resident in SBUF
        cht = const.tile([128, ND * K], fw_dt)
        nc.sync.dma_start(out=cht[:], in_=ch_p[:])
        if mode == "safe":
            cl16t = const.tile([128, ND * K], F16)
            ch16t = const.tile([128, ND * K], F16)
            nc.sync.dma_start(out=cl16t[:], in_=cl16_p[:])
            nc.sync.dma_start(out=ch16t[:], in_=ch16_p[:])
        elif mode == "fast":
            c8t = const.tile([128, 2 * ND * K], F8E5)
            nc.sync.dma_start(out=c8t[:], in_=c8_p[:])

        if bias_mode == "pe":
            nbt = const.tile([3, K], fw_dt)
            nc.sync.dma_start(out=nbt[:], in_=nb_p[:])
            ones3 = const.tile([3, 128], F16 if w16 else F32)
            nc.vector.memset(ones3[:], 1.0)
        else:
            nb1 = const.tile([1, K], F32)
            nc.sync.dma_start(out=nb1[:], in_=nb_p[:])
            ncsq_b = const.tile([128, K], F32)
            nc.gpsimd.partition_broadcast(ncsq_b[:], nb1[:])

        idx8 = const.tile([128, n_tiles * 8], U32, tag="idx8")
        fbuf = const.tile([128, n_tiles], F32)
        if mode == "flag":
            m018 = const.tile([128, n_tiles * 8], F32, tag="m018")

        loop_ctx = tc.For_i(0, repeat, 1) if repeat > 1 else nullcontext()
        with loop_ctx:
            for rt in range(n_tiles):
                rs = slice(rt * 128, (rt + 1) * 128)
                ft = work.tile([128, D], fw_dt, tag="ft")
                nc.sync.dma_start(out=ft[:], in_=fh_p[rs, :])
                if mode == "safe":
                    f16t = work.tile([128, 2 * D], F16, tag="f16t")
                    nc.sync.dma_start(out=f16t[:], in_=f16_p[rs, :])
                elif mode == "fast":
                    f8t = work.tile([128, 2 * D], F8E5, tag="f8t")
                    nc.sync.dma_start(out=f8t[:], in_=f8_p[rs, :])

                mp = psB.tile([128, K], F32, tag="mp")
                # main pass: h_f @ h_c, d-outer so each weight loads once.
                # wide=True streams all K=1024 columns per matmul (legal for
                # 16-bit moving operands; output spans two PSUM banks).
                kh_chunks = 1 if wide else 2
                kw = K // kh_chunks
                for d in range(ND):
                    for kh in range(kh_chunks):
                        ks = slice(kh * kw, (kh + 1) * kw)
                        nc.tensor.matmul(
                            mp[:, ks],
                            lhsT=ft[:, d * 128:(d + 1) * 128],
                            rhs=cht[:, d * K + kh * kw:d * K + (kh + 1) * kw],
                            start=d == 0, stop=False)
                if mode == "safe":
                    # corrections: fp16(f) @ fp16(l_c), then fp16(l_f) @ fp16(c)
                    for q, cv in enumerate((cl16t, ch16t)):
                        for d in range(ND):
                            for kh in range(2):
                                ks = slice(kh * 512, (kh + 1) * 512)
                                last = q == 1 and d == ND - 1
                                nc.tensor.matmul(
                                    mp[:, ks],
                                    lhsT=f16t[:, q * D + d * 128:q * D + (d + 1) * 128],
                                    rhs=cv[:, d * K + kh * 512:d * K + (kh + 1) * 512],
                                    start=False,
                                    stop=last and bias_mode != "pe")
                elif mode == "fast":
                    # fp8e5m2 DoubleRow: e5m2(f)@e5m2(l_c) + e5m2(l_f)@e5m2(c)
                    for d in range(ND):
                        lw = f8t[:, 2 * d * 128:(2 * d + 2) * 128].rearrange(
                            "p (q n) -> p q n", q=2)
                        cw = c8t[:, 2 * d * K:(2 * d + 2) * K].rearrange(
                            "p (q h k) -> p q h k", q=2, h=2)
                        for kh in range(2):
                            ks = slice(kh * 512, (kh + 1) * 512)
                            last = d == ND - 1
                            nc.tensor.matmul(
                                mp[:, ks],
                                lhsT=lw,
                                rhs=cw[:, :, kh, :],
                                perf_mode=mybir.MatmulPerfMode.DoubleRow,
                                start=False,
                                stop=last and bias_mode != "pe")
                if bias_mode == "pe":
                    # bias -|c|^2 as a 3-row matmul: [1,1,1] x [nh;nl;nll]
                    for kh in range(kh_chunks):
                        ks = slice(kh * kw, (kh + 1) * kw)
                        nc.tensor.matmul(
                            mp[:, ks],
                            lhsT=ones3[:] if w16 else ones3[:].bitcast(F32R),
                            rhs=nbt[:, ks],
                            start=False, stop=True)
                    sarg = mp
                else:
                    # GPSIMD has no PSUM port: Act copies PSUM->SBUF, Pool adds
                    s0 = work.tile([128, K], F32, tag="s0")
                    nc.scalar.copy(out=s0[:], in_=mp[:])
                    s = work.tile([128, K], F32, tag="s")
                    nc.gpsimd.tensor_tensor(out=s[:], in0=s0[:], in1=ncsq_b[:],
                                            op=mybir.AluOpType.add)
                    sarg = s

                if mode == "flag":
                    mv = m018[:, rt * 8:(rt + 1) * 8]
                else:
                    mv = red.tile([128, 8], F32, tag="mv")[:]
                nc.vector.max(mv, sarg[:])
                nc.vector.max_index(idx8[:, rt * 8:(rt + 1) * 8], mv, sarg[:])

        # gather col 0 of each 8-block, cast u32 -> f32, store
        nc.vector.tensor_copy(out=fbuf[:], in_=idx8[:, 0:n_tiles * 8:8])
        nc.sync.dma_start(out=out[:, 0].rearrange("(t p) -> p t", p=128),
                          in_=fbuf[:])
        if mode == "flag":
            nc.sync.dma_start(out=out[:, 1].rearrange("(t p) -> p t", p=128),
                              in_=m018[:, 0:n_tiles * 8:8])
            nc.sync.dma_start(out=out[:, 2].rearrange("(t p) -> p t", p=128),
                              in_=m018[:, 1:n_tiles * 8:8])

    nc.finalize()
    return nc


def _rne12(x: np.ndarray) -> np.ndarray:
    """Round fp32 to 12-bit significand (exactly representable in FP22)."""
    u = np.ascontiguousarray(x, dtype=np.float32).view(np.uint32)
    r = (u + np.uint32(0x7FF) + ((u >> np.uint32(12)) & np.uint32(1))) \
        & np.uint32(0xFFFFF000)
    return r.view(np.float32)


def _tile_rows(x: np.ndarray) -> np.ndarray:
    """[R, D] row-major -> [R, D] where row t*128+p holds (d = a*128+p)-major
    data for rows t*128+n: out[t*128+p, a*128+n] = x[t*128+n, a*128+p]."""
    r = x.shape[0]
    t = r // 128
    return np.ascontiguousarray(
        x.reshape(t, 128, ND, 128).transpose(0, 3, 2, 1)).reshape(r, D)


def _tile_cols(x: np.ndarray) -> np.ndarray:
    """[D, K] -> [128, ND*K]: out[p, a*K + k] = x[a*128 + p, k]."""
    return np.ascontiguousarray(
        x.reshape(ND, 128, K).transpose(1, 0, 2)).reshape(128, ND * K)


def prep_inputs(features: np.ndarray, centroids: np.ndarray,
                mode: str = None, bias_mode: str = None, w16: bool = None):
    """Host-side operand prep; returns per-core in_maps for build_bass."""
    import ml_dtypes
    mode = SHIP_KW["mode"] if mode is None else mode
    bias_mode = SHIP_KW["bias_mode"] if bias_mode is None else bias_mode
    w16 = SHIP_KW.get("w16", False) if w16 is None else w16
    f = np.ascontiguousarray(np.asarray(features, dtype=np.float32))
    c = np.ascontiguousarray(np.asarray(centroids, dtype=np.float32))
    c2 = (2.0 * c).astype(np.float32)

    fh = _rne12(f)
    fl = (f - fh).astype(np.float32)
    ch = _rne12(c2)
    cl = (c2 - ch).astype(np.float32)

    fh_t = _tile_rows(f).astype(np.float16) if w16 else _tile_rows(fh)
    ch_t = _tile_cols(c2).astype(np.float16) if w16 else _tile_cols(ch)

    common = {"ch": ch_t}
    if mode == "flag":
        pass
    elif mode == "safe":
        f16pair = np.empty((N, 2 * D), dtype=np.float16)
        f16pair[:, :D] = _tile_rows(f.astype(np.float16).astype(np.float32))
        f16pair[:, D:] = _tile_rows(fl.astype(np.float16).astype(np.float32))
        common["cl16"] = _tile_cols(cl.astype(np.float16).astype(np.float32)
                                    ).astype(np.float16)
        common["ch16"] = _tile_cols(c2.astype(np.float16).astype(np.float32)
                                    ).astype(np.float16)
    else:
        e5 = ml_dtypes.float8_e5m2
        f8pair = np.empty((N, 2 * D), dtype=e5)
        fh8 = f.astype(e5).astype(np.float32)
        fl8 = fl.astype(e5).astype(np.float32)
        ft8 = _tile_rows(fh8)  # q=0: e5m2(f)
        lt8 = _tile_rows(fl8)  # q=1: e5m2(l_f)
        for a in range(ND):
            f8pair[:, (2 * a) * 128:(2 * a + 1) * 128] = \
                ft8[:, a * 128:(a + 1) * 128].astype(e5)
            f8pair[:, (2 * a + 1) * 128:(2 * a + 2) * 128] = \
                lt8[:, a * 128:(a + 1) * 128].astype(e5)
        c8pair = np.empty((128, 2 * ND * K), dtype=e5)
        cl8 = _tile_cols(cl.astype(e5).astype(np.float32))   # q=0: e5m2(l_c)
        ch8 = _tile_cols(c2.astype(e5).astype(np.float32))   # q=1: e5m2(c)
        for a in range(ND):
            c8pair[:, (2 * a) * K:(2 * a + 1) * K] = \
                cl8[:, a * K:(a + 1) * K].astype(e5)
            c8pair[:, (2 * a + 1) * K:(2 * a + 2) * K] = \
                ch8[:, a * K:(a + 1) * K].astype(e5)

    ncsq = -(c.astype(np.float64) ** 2).sum(0, keepdims=True).astype(np.float32)
    if bias_mode == "pe":
        if w16:
            nh = ncsq.astype(np.float16).astype(np.float32)
            nl = (ncsq - nh).astype(np.float16).astype(np.float32)
            nll = (ncsq - nh - nl).astype(np.float16)
            common["nb"] = np.concatenate(
                [nh.astype(np.float16), nl.astype(np.float16), nll], axis=0)
        else:
            nh = _rne12(ncsq)
            nl = _rne12((ncsq - nh).astype(np.float32))
            nll = _rne12((ncsq - nh - nl).astype(np.float32))
            common["nb"] = np.concatenate([nh, nl, nll], axis=0)
    else:
        common["nb"] = ncsq

    in_maps = []
    for cc in range(N_CORES):
        rs = slice(cc * N_PER_CORE, (cc + 1) * N_PER_CORE)
        m = {"fh": fh_t[rs]}
        if mode == "safe":
            m["f16"] = f16pair[rs]
        elif mode == "fast":
            m["f8"] = f8pair[rs]
        m.update(common)
        in_maps.append(m)
    return in_maps


def _get_nc():
    if "nc" not in _cached:
        _cached["nc"] = build_bass(**SHIP_KW)
    return _cached["nc"]


def fixup_flagged(out3: np.ndarray, features: np.ndarray,
                  centroids: np.ndarray) -> np.ndarray:
    """Resolve near-tie rows (device top-2 gap < FLAG_T) exactly in fp64."""
    idx = out3[:, 0].copy()
    gap = out3[:, 1] - out3[:, 2]
    rows = np.flatnonzero(gap < FLAG_T)
    if len(rows):
        f64 = np.asarray(features, dtype=np.float64)[rows]
        c64 = np.asarray(centroids, dtype=np.float64)
        s = 2.0 * (f64 @ c64) - (c64 * c64).sum(0)
        idx[rows] = s.argmax(1).astype(np.float32)
    return idx[:, None].astype(np.float32)


def kernel(features: np.ndarray, centroids: np.ndarray) -> np.ndarray:
    in_maps = prep_inputs(features, centroids)
    nc = _get_nc()
    res = run_bass_kernel_spmd(nc, in_maps, list(range(N_CORES))).results
    out = np.concatenate([res[c]["out"] for c in range(N_CORES)], axis=0)
    if SHIP_KW["mode"] == "flag":
        return fixup_flagged(out, features, centroids)
    return out.astype(np.float32)


def _self_test():
    rng = np.random.default_rng(0)
    f = rng.standard_normal((N, D)).astype(np.float32)
    c = rng.standard_normal((D, K)).astype(np.float32)
    out = kernel(f, c)
    x = f @ c
    ref = (-2 * x + (c * c).sum(0)).argmin(1)
    print("mismatch:", (out[:, 0] != ref).sum(), "/", N)


if __name__ == "__main__":
    _self_test()
